# revision 1
# baseline (speedup 1.0000x reference)
"""Trainium2 Bass kernel for the DPAAUser3D segment-reduce problem.

Computes, for x[B=2,C=8,D=H=W=128] and attentions[B,C,512,1]:
  onehot = one_hot(argmax_c x)                      (per-voxel channel argmax)
  adj    = avgpool_8x8x8(onehot)                    ([B,C,16,16,16], = counts/512)
  corr[b,c,D,H,W] = att[b,c,(D//16*8+H//16)*8+W//16] * adj[b,c,D%16,H%16,W%16]
  out1   = x * (1+corr)^2
  out2   = corr

Sharding: data-parallel over the D axis (16 slices per core, 8 cores). The
argmax and pooling blocks are D-local, so each core computes its two pooled
kd-blocks exactly; one 16KB AllGather per batch element distributes the full
pooled count map to every core for the correction phase.

Phase 1 processes (b,d) slabs with H on partitions (needed by the pooling
matmul which contracts over H). Phase 2 re-reads x with partitions mapped to
(kd, H//16) so every DMA (x in, out1/out2 out) runs in contiguous 8KB bursts.
"""

import sys

import numpy as np

try:
    import concourse.bass as bass
except ImportError:  # fresh grading dir: concourse lives in the repo checkout
    for p in ("/opt/trn_rl_repo", "/root/.axon_site/_ro/trn_rl_repo"):
        if p not in sys.path:
            sys.path.insert(0, p)
    import concourse.bass as bass

import ml_dtypes
import concourse.bacc as bacc
import concourse.mybir as mybir
import concourse.tile as tile
from concourse.tile import add_dep_helper
from concourse import bass_utils

B, C, D, H, W = 2, 8, 128, 128, 128
POOL = 8          # pooling block edge
PATCH = 16        # fold patch edge
G = D // PATCH    # 8 patches per spatial dim
NCORES = 8
DL = D // NCORES  # 16 d-slices per core
PD = DL // POOL   # 2 pooled kd-blocks per core

F32 = mybir.dt.float32
BF16 = mybir.dt.bfloat16

_CACHE = {}


def _build_nc():
    nc = bacc.Bacc("TRN2", target_bir_lowering=False, debug=False,
                   num_devices=NCORES)

    xs = nc.dram_tensor("xs", [B, C, DL, H, W], F32, kind="ExternalInput").ap()
    # arep[b,c,q,wp] = att[b,c, core*64 + (q%8)*8 + wp] / 512  (q = kd*8+hp)
    arep = nc.dram_tensor("arep", [B, C, 128, G], F32, kind="ExternalInput").ap()
    pmat = nc.dram_tensor("pmat", [H, PATCH], BF16, kind="ExternalInput").ap()
    o1 = nc.dram_tensor("o1", [B, C, DL, H, W], F32, kind="ExternalOutput").ap()
    o2 = nc.dram_tensor("o2", [B, C, DL, H, W], F32, kind="ExternalOutput").ap()

    FS = C * PATCH * PATCH  # 2048: per-kd free size of the pooled-count map

    with tile.TileContext(nc) as tc:
        with (
            tc.tile_pool(name="big", bufs=1) as big,
            tc.tile_pool(name="p1", bufs=3) as p1,
            tc.tile_pool(name="p2", bufs=3) as p2,
            tc.tile_pool(name="psum", bufs=1, space="PSUM") as pp,
            tc.tile_pool(name="dram", bufs=1, space="DRAM") as dram,
        ):
            Pm = big.tile([128, PATCH], BF16, name="Pm")
            Ar = big.tile([128, B, C, G], F32, name="Ar")
            # AdjR[q, b, (c,kh,kw)]: pooled counts, kd=q//8 replicated over hp
            AdjR = big.tile([128, B, FS], F32, name="AdjR")

            nc.sync.dma_start(out=Pm, in_=pmat)
            for b in range(B):
                nc.sync.dma_start(out=Ar[:, b], in_=arep[b].transpose([1, 0, 2]))

            psums = {}
            for b in range(B):
                for pd in range(PD):
                    for hf in range(2):
                        t = pp.tile([16, 512], F32, name=f"ps{b}{pd}{hf}",
                                    tag=f"ps{b}{pd}{hf}")
                        psums[(b, pd, hf)] = t

            adj_in = [dram.tile([PD, C, 16, 16], F32, name=f"adj_in{b}")
                      for b in range(B)]
            adj_gat = [dram.tile([NCORES, PD, C, 16, 16], F32,
                                 name=f"adj_gat{b}", addr_space="Shared")
                       for b in range(B)]

            # ---- phase 1: argmax one-hot + pooled counts ----
            last_p1_dve = None
            last_slab_load = None
            for b in range(B):
                for d in range(DL):
                    slab = p1.tile([128, C, W], F32, name="slab", tag="slab")
                    last_slab_load = nc.sync.dma_start(
                        out=slab, in_=xs[b, :, d].transpose([1, 0, 2]))
                    t1 = p1.tile([128, 4, W], F32, name="t1", tag="t1")
                    nc.vector.tensor_max(t1, slab[:, 0:4, :], slab[:, 4:8, :])
                    t2 = p1.tile([128, 2, W], F32, name="t2", tag="t2")
                    nc.vector.tensor_max(t2, t1[:, 0:2, :], t1[:, 2:4, :])
                    M = p1.tile([128, W], F32, name="M", tag="M")
                    nc.vector.tensor_max(M, t2[:, 0, :], t2[:, 1, :])
                    eq = p1.tile([128, C, W], BF16, name="eq", tag="eq")
                    nc.vector.tensor_tensor(
                        eq, slab, M.unsqueeze(1).broadcast_to([128, C, W]),
                        op=mybir.AluOpType.is_equal)
                    eqf = eq.rearrange("p c w -> p (c w)")
                    pd, dd = d // POOL, d % POOL
                    for hf in range(2):
                        nc.tensor.matmul(psums[(b, pd, hf)], lhsT=Pm,
                                         rhs=eqf[:, hf * 512:(hf + 1) * 512],
                                         start=(dd == 0), stop=(dd == POOL - 1))
                    if dd == POOL - 1:
                        adjp = p1.tile([16, C, 16], F32, name="adjp", tag="adjp")
                        for hf in range(2):
                            src = psums[(b, pd, hf)].rearrange(
                                "p (c wb wi) -> p c wb wi", c=4, wb=16, wi=8)
                            last_p1_dve = nc.vector.reduce_sum(
                                adjp[:, hf * 4:(hf + 1) * 4, :], src,
                                axis=mybir.AxisListType.X)
                        # payload [pd][c, ph, pw]; on the scalar ring (idle
                        # until phase 2) so neither the sync ring nor the
                        # gpsimd collective stream stalls behind this DMA's
                        # DVE-reduce dependency
                        nc.scalar.dma_start(out=adj_in[b][pd].transpose([1, 0, 2]),
                                            in_=adjp)
                # per-b AllGather: fires mid-kernel, overlaps remaining work
                nc.gpsimd.collective_compute(
                    "AllGather", mybir.AluOpType.bypass,
                    replica_groups=[list(range(NCORES))],
                    ins=[adj_in[b].opt()], outs=[adj_gat[b].opt()])
                # gathered [core,pd,c,ph,pw] flat == [kd, (c,kh,kw)]; load with
                # 8x partition replication: q = kd*8 + hp reads row kd = q//8.
                # On the gpsimd stream, which is already blocked on this
                # AllGather; sync/scalar rings keep flowing.
                rep = bass.AP(tensor=adj_gat[b].tensor, offset=adj_gat[b].offset,
                              ap=[[FS, DL], [0, POOL], [1, FS]])
                nc.gpsimd.dma_start(out=AdjR[:, b], in_=rep)

            # ---- phase 2: correction + outputs (partitions = (kd, hp)) ----
            for b in range(B):
                for c in range(C):
                    xv = xs[b, c].rearrange("d (a k) w -> (d a) (k w)", a=POOL)
                    x2 = p2.tile([128, PATCH * W], F32, name="x2", tag="x2",
                                 bufs=4)
                    x2_ld = nc.sync.dma_start(out=x2, in_=xv)
                    # keep the sync ring draining phase-1 slab loads first
                    add_dep_helper(x2_ld.ins, last_slab_load.ins, False,
                                   "phase-1 loads first")
                    corr = p2.tile([128, PATCH, G, PATCH], F32, name="corr",
                                   tag="corr")
                    a_b = Ar[:, b, c].unsqueeze(1).unsqueeze(3).broadcast_to(
                        [128, PATCH, G, PATCH])
                    r_b = AdjR[:, b].rearrange(
                        "p (c kh kw) -> p c kh kw", c=C, kh=PATCH)[:, c] \
                        .unsqueeze(2).broadcast_to([128, PATCH, G, PATCH])
                    corr_i = nc.vector.tensor_mul(corr, a_b, r_b)
                    # DVE must finish all phase-1 work before phase-2; without
                    # this the scheduler can park DVE on corr (blocked on the
                    # AllGather) while ready phase-1 slabs starve behind it
                    add_dep_helper(corr_i.ins, last_p1_dve.ins, False,
                                   "phase-1 DVE first")
                    corr_f = corr.rearrange("p a g k -> p (a g k)")
                    u2 = p2.tile([128, PATCH * W], F32, name="u2", tag="u2",
                                 bufs=3)
                    nc.scalar.activation(u2, corr_f,
                                         mybir.ActivationFunctionType.Square,
                                         bias=1.0, scale=1.0)
                    o1t = p2.tile([128, PATCH * W], F32, name="o1t", tag="o1t",
                                  bufs=3)
                    nc.vector.tensor_mul(o1t, x2, u2)
                    ov1 = o1[b, c].rearrange("d (a k) w -> (d a) (k w)", a=POOL)
                    ov2 = o2[b, c].rearrange("d (a k) w -> (d a) (k w)", a=POOL)
                    nc.scalar.dma_start(out=ov2, in_=corr_f)
                    nc.sync.dma_start(out=ov1, in_=o1t)

    nc.compile()
    return nc


def _fix_ties(x):
    """The device one-hot marks every channel equal to the max; the reference
    one_hot(argmax) marks only the first. Nudge later tied channels down by
    one ulp so a plain equality compare reproduces first-match semantics
    (out1 changes by <=1 ulp at those voxels)."""
    mx = x.max(axis=1, keepdims=True)
    ties = x == mx
    multi = ties.sum(axis=1) > 1
    if not multi.any():
        return x
    x = x.copy()
    for b, d, h, w in np.argwhere(multi):
        cs = np.flatnonzero(ties[b, :, d, h, w])
        for c in cs[1:]:
            x[b, c, d, h, w] = np.nextafter(x[b, c, d, h, w], -np.inf)
    return x


def _host_inputs(x, attentions):
    """Build per-core input maps from full inputs."""
    x = _fix_ties(x)
    att = attentions[..., 0].astype(np.float32) * np.float32(1.0 / 512.0)
    att_p = att.reshape(B, C, G, G, G)  # [b, c, dp, hp, wp]
    pm = np.zeros((H, PATCH), dtype=ml_dtypes.bfloat16)
    pm[np.arange(H), np.arange(H) // POOL] = 1.0

    in_maps = []
    for core in range(NCORES):
        xs = np.ascontiguousarray(x[:, :, core * DL:(core + 1) * DL])
        # arep[b,c,q,wp] = att_p[b,c,core, q%8, wp]  (q = kd*8 + hp)
        arep = np.ascontiguousarray(
            np.tile(att_p[:, :, core], (1, 1, DL, 1)).reshape(B, C, 128, G))
        in_maps.append({"xs": xs, "arep": arep, "pmat": pm})
    return in_maps


def kernel(x, attentions):
    x = np.asarray(x, dtype=np.float32)
    attentions = np.asarray(attentions, dtype=np.float32)

    if "nc" not in _CACHE:
        _CACHE["nc"] = _build_nc()
    nc = _CACHE["nc"]

    in_maps = _host_inputs(x, attentions)
    res = bass_utils.run_bass_kernel_spmd(nc, in_maps,
                                          core_ids=list(range(NCORES)))

    out1 = np.empty((B, C, D, H, W), np.float32)
    out2 = np.empty((B, C, D, H, W), np.float32)
    for core in range(NCORES):
        out1[:, :, core * DL:(core + 1) * DL] = res.results[core]["o1"]
        out2[:, :, core * DL:(core + 1) * DL] = res.results[core]["o2"]
    return out1, out2

